# revision 37
# baseline (speedup 1.0000x reference)
"""BevPoolV2 Trainium2 kernel (8 NeuronCores, SPMD, no collectives).

v4: multi-queue SWDGE (gathers spread over Q7 core pairs 0-3), batched
DVE depth extraction (is_equal+mult+reduce over half-chunks instead of
per-tile scalar_tensor_tensor), merged scatters (2 per chunk, even/odd
tiles), deeper tile-pool pipelining, B-stream gathers hoisted ahead of
the A loop.
v3: f32-view gathers (bitcast bf16 for compute).
v2: bf16 gather/matmul path, CHUNK=64, psum groups.
Structure: sorted point stream cut at BEV-cell boundaries; core k owns
cells [4096k, 4096(k+1)); window matmuls accumulate into a [4097, 128]
DRAM slab via chained dma_scatter_add; stream B handles feat rows >=
32768 (int16 gather index limit).
"""
import numpy as np

B, N, D, H, W = 2, 6, 120, 32, 88
C = 128
NCELLS = 32768
NCORES = 8
CELLS_PER_CORE = NCELLS // NCORES   # 4096
TILE_P = 128
WIN = 8
CHUNK = 64                          # tiles per A-chunk
NCHUNK = 15
HALF = 32                           # tiles per psum half-chunk
T_A = NCHUNK * CHUNK                # 960 A-tiles
DUMMY = CELLS_PER_CORE              # trash row 4096
N_FEAT_ROWS = B * N * H * W         # 33792
N_DEPTH = B * N * D * H * W         # 4055040
N_DEP_BLK = N_DEPTH // 128          # 31680
A_LIM = 32768                       # feat rows handled by stream A
NB_ROWS = N_FEAT_ROWS - A_LIM       # 1024 rows in stream-B table view
B_CAPS = (21, 9, 3, 1, 1, 1, 1, 1)  # blocks per B scatter call
NB_BLOCKS = sum(B_CAPS)             # 36
NB_SLOTS = NB_BLOCKS * 128          # 4608


def _pack16(ent):
    """entry i -> int16 storage [i%16, i//16], replicated to 128 partitions."""
    a = np.asarray(ent, np.int16).reshape(-1, 16).T
    return np.ascontiguousarray(np.tile(a, (8, 1)))


def _bf16(x):
    import ml_dtypes
    return np.ascontiguousarray(np.asarray(x).astype(ml_dtypes.bfloat16))


# ---------------------------------------------------------------- host prep
def _preprocess(ranks_depth, ranks_feat, ranks_bev):
    ranks_bev = np.asarray(ranks_bev)
    ranks_feat = np.asarray(ranks_feat).astype(np.int64)
    ranks_depth = np.asarray(ranks_depth).astype(np.int64)
    bounds = np.searchsorted(ranks_bev, np.arange(0, NCELLS + 1, CELLS_PER_CORE))
    cores = []
    for k in range(NCORES):
        lo, hi = int(bounds[k]), int(bounds[k + 1])
        rb = ranks_bev[lo:hi].astype(np.int64) - k * CELLS_PER_CORE
        rf = ranks_feat[lo:hi]
        rd = ranks_depth[lo:hi]
        isB = rf >= A_LIM

        # ---------------- stream A ----------------
        rbA, rfA, rdA = rb[~isB], rf[~isB], rd[~isB]
        n = len(rbA)
        assert np.bincount(rbA, minlength=1).max() < 2 * TILE_P
        tiles = []
        s = 0
        while s < n:
            e = min(s + TILE_P, n)
            cut = s + int(np.searchsorted(rbA[s:e], rbA[s] + WIN))
            e = min(e, cut) if cut > s else e
            tiles.append((s, e))
            s = e
        nt = len(tiles)
        assert nt <= T_A, (k, nt)

        rf_i = np.zeros((T_A, TILE_P), np.int64)
        rd_i = np.zeros((T_A, TILE_P), np.int64)
        mask = np.zeros((T_A, TILE_P, WIN), np.float32)
        win0 = np.zeros(T_A, np.int64)
        wid = np.zeros(T_A, np.int64)
        for t, (ts, te) in enumerate(tiles):
            m = te - ts
            rf_i[t, :m] = rfA[ts:te]
            rd_i[t, :m] = rdA[ts:te]
            col = rbA[ts:te] - rbA[ts]
            mask[t, np.arange(m), col] = 1.0
            win0[t] = rbA[ts]
            wid[t] = col[-1] + 1

        rfiA = np.empty((NCHUNK, TILE_P, CHUNK * TILE_P // 16), np.int16)
        rdiA = np.empty_like(rfiA)
        mskA = np.empty((NCHUNK, TILE_P, CHUNK * WIN), np.float32)
        rdmA = np.empty((NCHUNK, TILE_P, CHUNK), np.float32)
        for c in range(NCHUNK):
            t0 = c * CHUNK
            rfiA[c] = _pack16(rf_i[t0:t0 + CHUNK].reshape(-1))
            rdiA[c] = _pack16((rd_i[t0:t0 + CHUNK] // 128).reshape(-1))
            mskA[c] = mask[t0:t0 + CHUNK].transpose(1, 0, 2).reshape(
                TILE_P, CHUNK * WIN)
            rdmA[c] = (rd_i[t0:t0 + CHUNK] % 128).astype(np.float32).T

        # merged scatter: 2 calls per chunk (even/odd tiles), 256 entries
        # each.  Entry i of call `par`: P=i%128, h=i//128,
        # j = 32*h + 2*(P%16) + par, w = P//16.
        sidxA = np.empty((NCHUNK, TILE_P, 2 * 16), np.int16)
        i_arr = np.arange(256)
        P_arr, h_arr = i_arr % 128, i_arr // 128
        w_arr = P_arr // 16
        for c in range(NCHUNK):
            t0 = c * CHUNK
            for par in range(2):
                j = 32 * h_arr + 2 * (P_arr % 16) + par
                t = t0 + j
                ent = np.where(w_arr < wid[t], win0[t] + w_arr,
                               DUMMY + i_arr)
                sidxA[c, :, 16 * par:16 * par + 16] = _pack16(ent)
        # scatter calls are unchained: cells must be globally unique per
        # parity slab (adjacent-tile window overlap always crosses parity)
        for par in range(2):
            j = (32 * h_arr[None, :] + 2 * (P_arr[None, :] % 16) + par
                 + CHUNK * np.arange(NCHUNK)[:, None])
            ent = np.where(w_arr[None, :] < wid[j], win0[j] + w_arr[None, :],
                           DUMMY).ravel()
            live = ent[ent != DUMMY]
            assert len(live) == len(np.unique(live)), (k, par)

        # ---------------- stream B ----------------
        rbB, rfB, rdB = rb[isB], rf[isB], rd[isB]
        nB = len(rbB)
        assert nB <= NB_SLOTS, (k, nB)
        starts = np.concatenate([[0], np.cumsum(B_CAPS)]) * 128  # slot starts
        fill = list(starts[:-1])                  # next free slot per bin
        cell_bins = {}                            # cell -> set of bins used
        slot_of = np.full(NB_SLOTS, -1, np.int64)  # slot -> B-point index
        for i in range(nB):
            cell = int(rbB[i])
            used = cell_bins.setdefault(cell, set())
            placed = False
            for o in range(len(B_CAPS)):
                if o in used or fill[o] >= starts[o + 1]:
                    continue
                slot_of[fill[o]] = i
                fill[o] += 1
                used.add(o)
                placed = True
                break
            assert placed, (k, i, cell)
        rfiB_e = np.zeros(NB_SLOTS, np.int64)
        rdiB_e = np.zeros(NB_SLOTS, np.int64)
        rdmB = np.zeros((TILE_P, NB_BLOCKS), np.float32)
        srowB = DUMMY + (np.arange(NB_SLOTS) % 512)
        occ_s = slot_of >= 0
        pi = slot_of[occ_s]
        rfiB_e[occ_s] = rfB[pi] - A_LIM
        rdiB_e[occ_s] = rdB[pi] // 128
        srowB[occ_s] = rbB[pi]
        g = np.arange(NB_SLOTS)
        rdm_flat = np.zeros(NB_SLOTS, np.float32)
        rdm_flat[occ_s] = (rdB[pi] % 128).astype(np.float32)
        rdmB[g % 128, g // 128] = rdm_flat
        rfiB = _pack16(rfiB_e)
        rdiB = _pack16(rdiB_e)
        sidxB = np.empty((TILE_P, NB_BLOCKS * WIN), np.int16)
        for o in range(len(B_CAPS)):
            s0, e0 = starts[o], starts[o + 1]
            sidxB[:, (s0 // 16):(e0 // 16)] = _pack16(srowB[s0:e0])

        cores.append(dict(rfiA=rfiA, rdiA=rdiA, mskA=_bf16(mskA),
                          rdmA=_bf16(rdmA), sidxA=sidxA, rfiB=rfiB,
                          rdiB=rdiB, rdmB=_bf16(rdmB), sidxB=sidxB))
    return cores


# ---------------------------------------------------------------- program
_CACHED = {}


def _build_program():
    import concourse.bass as bass
    import concourse.bacc as bacc
    import concourse.tile as tile
    from concourse import mybir
    from concourse.tile import add_dep_helper

    nc = bacc.Bacc("TRN2", target_bir_lowering=False, debug=False,
                   num_swdge_queues=4)
    f32, bf16, i16 = mybir.dt.float32, mybir.dt.bfloat16, mybir.dt.int16
    NI = CHUNK * TILE_P                 # 8192 idxs per A chunk
    feat_t = nc.dram_tensor("feat_tbl", [N_FEAT_ROWS, C // 2], f32,
                            kind="ExternalInput").ap()
    dep_t = nc.dram_tensor("dep_tbl", [N_DEP_BLK, 64], f32,
                           kind="ExternalInput").ap()
    iota_t = nc.dram_tensor("iota", [TILE_P, 128], bf16,
                            kind="ExternalInput").ap()
    rfiA_t = nc.dram_tensor("rfiA", [NCHUNK, TILE_P, NI // 16], i16,
                            kind="ExternalInput").ap()
    rdiA_t = nc.dram_tensor("rdiA", [NCHUNK, TILE_P, NI // 16], i16,
                            kind="ExternalInput").ap()
    mskA_t = nc.dram_tensor("mskA", [NCHUNK, TILE_P, CHUNK * WIN], bf16,
                            kind="ExternalInput").ap()
    rdmA_t = nc.dram_tensor("rdmA", [NCHUNK, TILE_P, CHUNK], bf16,
                            kind="ExternalInput").ap()
    sidxA_t = nc.dram_tensor("sidxA", [NCHUNK, TILE_P, 2 * 16], i16,
                             kind="ExternalInput").ap()
    rfiB_t = nc.dram_tensor("rfiB", [TILE_P, NB_SLOTS // 16], i16,
                            kind="ExternalInput").ap()
    rdiB_t = nc.dram_tensor("rdiB", [TILE_P, NB_SLOTS // 16], i16,
                            kind="ExternalInput").ap()
    rdmB_t = nc.dram_tensor("rdmB", [TILE_P, NB_BLOCKS], bf16,
                            kind="ExternalInput").ap()
    sidxB_t = nc.dram_tensor("sidxB", [TILE_P, NB_BLOCKS * WIN], i16,
                             kind="ExternalInput").ap()
    out_t = nc.dram_tensor("out", [CELLS_PER_CORE + 512, C], f32,
                           kind="ExternalOutput").ap()
    out2_t = nc.dram_tensor("out2", [CELLS_PER_CORE + 512, C], f32,
                            kind="ExternalOutput").ap()
    out3_t = nc.dram_tensor("out3", [CELLS_PER_CORE + 512, C], f32,
                            kind="ExternalOutput").ap()

    EQ, MUL, ADD = (mybir.AluOpType.is_equal, mybir.AluOpType.mult,
                    mybir.AluOpType.add)
    AXX = mybir.AxisListType.X

    with tile.TileContext(nc) as tc:
        with (
            tc.tile_pool(name="cst", bufs=1) as cst,
            tc.tile_pool(name="seq", bufs=3) as seq,
            tc.tile_pool(name="gp", bufs=2) as gp,
            tc.tile_pool(name="dp", bufs=3) as dp,
            tc.tile_pool(name="sp", bufs=2) as sp,
            tc.tile_pool(name="xp", bufs=2) as xp,
            tc.tile_pool(name="sip", bufs=NCHUNK) as sip,
            tc.tile_pool(name="sg", bufs=NCHUNK) as sg,
            tc.tile_pool(name="ps", bufs=8, space="PSUM") as ps,
        ):
            iota_sb = cst.tile([TILE_P, 128], bf16)
            nc.sync.dma_start(iota_sb[:], iota_t)

            # ---- stream B gathers first (fills queues 1/2 during warmup)
            rfiB_sb = cst.tile([TILE_P, NB_SLOTS // 16], i16)
            rdiB_sb = cst.tile([TILE_P, NB_SLOTS // 16], i16)
            rdmB_sb = cst.tile([TILE_P, NB_BLOCKS], bf16)
            sidB_sb = cst.tile([TILE_P, NB_BLOCKS * WIN], i16)
            nc.sync.dma_start(rfiB_sb[:], rfiB_t)
            nc.sync.dma_start(rdiB_sb[:], rdiB_t)
            nc.sync.dma_start(rdmB_sb[:], rdmB_t)
            nc.sync.dma_start(sidB_sb[:], sidxB_t)

            gB_sb = cst.tile([TILE_P, NB_BLOCKS * C // 2], f32)
            dbB_sb = cst.tile([TILE_P, NB_BLOCKS * 64], f32)

            # ---- stream A ----
            stg_tiles = []
            prev = None
            for c in range(NCHUNK):
                rfi_sb = seq.tile([TILE_P, NI // 16], i16, tag="rfi")
                rdi_sb = seq.tile([TILE_P, NI // 16], i16, tag="rdi")
                msk_sb = seq.tile([TILE_P, CHUNK * WIN], bf16, tag="msk")
                rdm_sb = seq.tile([TILE_P, CHUNK], bf16, tag="rdm")
                sid_sb = sip.tile([TILE_P, 2 * 16], i16, tag="sid")
                nc.sync.dma_start(rfi_sb[:], rfiA_t[c])
                nc.sync.dma_start(rdi_sb[:], rdiA_t[c])
                nc.sync.dma_start(msk_sb[:], mskA_t[c])
                nc.sync.dma_start(rdm_sb[:], rdmA_t[c])
                nc.sync.dma_start(sid_sb[:], sidxA_t[c])

                g_sb = gp.tile([TILE_P, CHUNK * C // 2], f32, tag="g")
                db_sb = gp.tile([TILE_P, CHUNK * 64], f32, tag="db")
                g3f = g_sb[:].rearrange("p (j e) -> p j e", e=C // 2)
                db3f = db_sb[:].rearrange("p (j e) -> p j e", e=64)
                HN = NI // 2
                nc.gpsimd.dma_gather(g3f[:, :HALF, :], feat_t,
                                     rfi_sb[:, :HN // 16], HN, HN, C // 2,
                                     single_packet=False,
                                     queue_num=c % 4)
                nc.gpsimd.dma_gather(g3f[:, HALF:, :], feat_t,
                                     rfi_sb[:, HN // 16:], HN, HN, C // 2,
                                     single_packet=False,
                                     queue_num=(c + 1) % 4)
                nc.gpsimd.dma_gather(db3f[:, :HALF, :], dep_t,
                                     rdi_sb[:, :HN // 16], HN, HN, 64,
                                     single_packet=False,
                                     queue_num=(c + 2) % 4)
                nc.gpsimd.dma_gather(db3f[:, HALF:, :], dep_t,
                                     rdi_sb[:, HN // 16:], HN, HN, 64,
                                     single_packet=False,
                                     queue_num=(c + 3) % 4)
                if c == 5:
                    gB3f = gB_sb[:].rearrange("p (j e) -> p j e", e=C // 2)
                    dbB3f = dbB_sb[:].rearrange("p (j e) -> p j e", e=64)
                    nc.gpsimd.dma_gather(gB3f, feat_t[A_LIM:, :],
                                         rfiB_sb[:], NB_SLOTS, NB_SLOTS,
                                         C // 2, single_packet=False,
                                         queue_num=(c + 1) % 4)
                if c == 6:
                    nc.gpsimd.dma_gather(dbB3f, dep_t, rdiB_sb[:],
                                         NB_SLOTS, NB_SLOTS, 64,
                                         single_packet=False,
                                         queue_num=(c + 2) % 4)

                g3 = g_sb[:].bitcast(bf16).rearrange("p (j e) -> p j e", e=C)
                db3 = db_sb[:].bitcast(bf16).rearrange(
                    "p (j e) -> p j e", e=128)

                # depth extraction: d[p, j] = db3[p, j, rdm[p, j]]
                d_sb = dp.tile([TILE_P, CHUNK], bf16, tag="d")
                iota3 = iota_sb[:].rearrange("p (j e) -> p j e", j=1) \
                    .to_broadcast([TILE_P, HALF, 128])
                for h in range(2):
                    sel_sb = sp.tile([TILE_P, HALF * 128], bf16, tag="sel")
                    sel3 = sel_sb[:].rearrange("p (j e) -> p j e", e=128)
                    rdm3 = rdm_sb[:, h * HALF:(h + 1) * HALF] \
                        .rearrange("p (j o) -> p j o", o=1) \
                        .to_broadcast([TILE_P, HALF, 128])
                    nc.vector.tensor_tensor(
                        out=sel3, in0=iota3, in1=rdm3, op=EQ)
                    nc.vector.tensor_tensor(
                        out=sel3, in0=sel3,
                        in1=db3[:, h * HALF:(h + 1) * HALF, :], op=MUL)
                    with nc.allow_low_precision(
                            reason="one-hot select, exact"):
                        nc.vector.tensor_reduce(
                            out=d_sb[:, h * HALF:(h + 1) * HALF], in_=sel3,
                            axis=AXX, op=ADD)

                ad_sb = dp.tile([TILE_P, CHUNK * WIN], bf16, tag="ad")
                ad3 = ad_sb[:].rearrange("p (j w) -> p j w", w=WIN)
                nc.vector.tensor_tensor(
                    out=ad3, in0=msk_sb[:].rearrange("p (j w) -> p j w", w=WIN),
                    in1=d_sb[:].to_broadcast([TILE_P, CHUNK, WIN]), op=MUL)

                # staging: element (w, j2, e) of half h lands at stg
                # partition 16w + j2//2, block 2*(j2%2) + h (flat-order DMA
                # pairing); even tiles end up in blocks 0-1, odd in 2-3.
                stg_sb = sg.tile([TILE_P, 4 * C], f32, tag="stg")
                st4 = stg_sb[:].rearrange("P (b e) -> P b e", e=C)
                for h in range(2):
                    tmp_sb = xp.tile([WIN, HALF * C], f32, tag="tmp")
                    for q in range(HALF // 4):
                        pt = ps.tile([WIN, 4 * C], f32, tag="pt", space="PSUM")
                        for m in range(4):
                            j = HALF * h + 4 * q + m
                            nc.tensor.matmul(out=pt[:, C * m:C * (m + 1)],
                                             lhsT=ad3[:, j, :], rhs=g3[:, j, :],
                                             start=True, stop=True)
                        nc.scalar.copy(tmp_sb[:, 4 * C * q:4 * C * (q + 1)],
                                       pt[:])
                    nc.scalar.dma_start(
                        st4[:, h::2, :],
                        tmp_sb[:].rearrange("w (j e) -> w j e", e=C))
                stg_tiles.append((stg_sb, sid_sb))

            # deferred unchained A scatters: even tiles -> out, odd -> out2
            for c in range(NCHUNK):
                stg_sb, sid_sb = stg_tiles[c]
                st4 = stg_sb[:].rearrange("P (b e) -> P b e", e=C)
                for par in range(2):
                    nc.gpsimd.dma_scatter_add(
                        (out_t, out2_t)[par], st4[:, 2 * par:2 * par + 2, :],
                        sid_sb[:, 16 * par:16 * par + 16],
                        256, 256, C, single_packet=True,
                        queue_num=(2 * c + par) % 4)

            # ---- stream B compute + scatter ----
            gB3 = gB_sb[:].bitcast(bf16).rearrange("p (j e) -> p j e", e=C)
            dbB3 = dbB_sb[:].bitcast(bf16).rearrange(
                "p (j e) -> p j e", e=128)
            dB_sb = cst.tile([TILE_P, NB_BLOCKS], bf16)
            iotaB3 = iota_sb[:].rearrange("p (j e) -> p j e", j=1) \
                .to_broadcast([TILE_P, NB_BLOCKS // 2, 128])
            for h in range(2):
                NBH = NB_BLOCKS // 2
                selB_sb = sp.tile([TILE_P, HALF * 128], bf16, tag="sel")
                selB3 = selB_sb[:].rearrange(
                    "p (j e) -> p j e", e=128)[:, :NBH, :]
                rdmB3 = rdmB_sb[:, h * NBH:(h + 1) * NBH] \
                    .rearrange("p (j o) -> p j o", o=1) \
                    .to_broadcast([TILE_P, NBH, 128])
                nc.vector.tensor_tensor(
                    out=selB3, in0=iotaB3, in1=rdmB3, op=EQ)
                nc.vector.tensor_tensor(
                    out=selB3, in0=selB3,
                    in1=dbB3[:, h * NBH:(h + 1) * NBH, :], op=MUL)
                with nc.allow_low_precision(
                        reason="one-hot select, exact"):
                    nc.vector.tensor_reduce(
                        out=dB_sb[:, h * NBH:(h + 1) * NBH], in_=selB3,
                        axis=AXX, op=ADD)
            gsB_sb = cst.tile([TILE_P, NB_BLOCKS * C], f32)
            gsB3 = gsB_sb[:].rearrange("p (j e) -> p j e", e=C)
            nc.vector.tensor_tensor(
                out=gsB3, in0=gB3,
                in1=dB_sb[:].to_broadcast([TILE_P, NB_BLOCKS, C]), op=MUL)

            s0 = 0
            prev = None
            for o, cap in enumerate(B_CAPS):
                e0 = s0 + cap
                sc = nc.gpsimd.dma_scatter_add(
                    out3_t, gsB3[:, s0:e0, :],
                    sidB_sb[:, WIN * s0:WIN * e0],
                    128 * cap, 128 * cap, C, single_packet=True,
                    queue_num=2)
                if prev is not None:
                    add_dep_helper(sc.ins, prev.ins, reason="B scatter chain")
                prev = sc
                s0 = e0
    nc.compile()
    return nc


def _get_program():
    if "nc" not in _CACHED:
        _CACHED["nc"] = _build_program()
    return _CACHED["nc"]


# ---------------------------------------------------------------- entry
def kernel(depth, feat, ranks_depth, ranks_feat, ranks_bev,
           interval_starts=None, interval_lengths=None):
    from concourse import bass_utils

    depth = np.asarray(depth, dtype=np.float32)
    feat = np.asarray(feat, dtype=np.float32)
    feat_flat = _bf16(feat.transpose(0, 1, 3, 4, 2).reshape(-1, C)) \
        .view(np.float32)
    dep_blk = _bf16(depth.reshape(N_DEP_BLK, 128)).view(np.float32)
    iota = _bf16(np.broadcast_to(np.arange(128, dtype=np.float32),
                                 (TILE_P, 128)))

    cores = _preprocess(ranks_depth, ranks_feat, ranks_bev)
    in_maps = []
    for k in range(NCORES):
        cd = cores[k]
        in_maps.append({
            "feat_tbl": feat_flat, "dep_tbl": dep_blk, "iota": iota,
            "rfiA": cd["rfiA"], "rdiA": cd["rdiA"], "mskA": cd["mskA"],
            "rdmA": cd["rdmA"], "sidxA": cd["sidxA"],
            "rfiB": cd["rfiB"], "rdiB": cd["rdiB"], "rdmB": cd["rdmB"],
            "sidxB": cd["sidxB"],
        })

    nc = _get_program()
    res = bass_utils.run_bass_kernel_spmd(nc, in_maps,
                                          core_ids=list(range(NCORES)))
    _CACHED["last_results"] = res

    out_full = np.zeros((B, C, 1, 128, 128), np.float32)
    for k in range(NCORES):
        r = res.results[k]
        oc = (np.asarray(r["out"]) + np.asarray(r["out2"])
              + np.asarray(r["out3"]))[:CELLS_PER_CORE]
        b, blk = k // 4, k % 4
        out_full[b, :, 0, 32 * blk:32 * (blk + 1), :] = \
            oc.T.reshape(C, 32, 128)
    return out_full


# revision 38
# speedup vs baseline: 1.0185x; 1.0185x over previous
"""BevPoolV2 Trainium2 kernel (8 NeuronCores, SPMD, no collectives).

v4: multi-queue SWDGE (gathers spread over Q7 core pairs 0-3), batched
DVE depth extraction (is_equal+mult+reduce over half-chunks instead of
per-tile scalar_tensor_tensor), merged scatters (2 per chunk, even/odd
tiles), deeper tile-pool pipelining, B-stream gathers hoisted ahead of
the A loop.
v3: f32-view gathers (bitcast bf16 for compute).
v2: bf16 gather/matmul path, CHUNK=64, psum groups.
Structure: sorted point stream cut at BEV-cell boundaries; core k owns
cells [4096k, 4096(k+1)); window matmuls accumulate into a [4097, 128]
DRAM slab via chained dma_scatter_add; stream B handles feat rows >=
32768 (int16 gather index limit).
"""
import numpy as np

B, N, D, H, W = 2, 6, 120, 32, 88
C = 128
NCELLS = 32768
NCORES = 8
CELLS_PER_CORE = NCELLS // NCORES   # 4096
TILE_P = 128
WIN = 8
CHUNK = 64                          # tiles per A-chunk
NCHUNK = 15
HALF = 32                           # tiles per psum half-chunk
T_A = NCHUNK * CHUNK                # 960 A-tiles
DUMMY = CELLS_PER_CORE              # trash row 4096
N_FEAT_ROWS = B * N * H * W         # 33792
N_DEPTH = B * N * D * H * W         # 4055040
N_DEP_BLK = N_DEPTH // 128          # 31680
A_LIM = 32768                       # feat rows handled by stream A
NB_ROWS = N_FEAT_ROWS - A_LIM       # 1024 rows in stream-B table view
B_CAPS = (21, 9, 3, 1, 1, 1, 1, 1)  # blocks per B scatter call
NB_BLOCKS = sum(B_CAPS)             # 36
NB_SLOTS = NB_BLOCKS * 128          # 4608


def _pack16(ent):
    """entry i -> int16 storage [i%16, i//16], replicated to 128 partitions."""
    a = np.asarray(ent, np.int16).reshape(-1, 16).T
    return np.ascontiguousarray(np.tile(a, (8, 1)))


def _bf16(x):
    import ml_dtypes
    return np.ascontiguousarray(np.asarray(x).astype(ml_dtypes.bfloat16))


# ---------------------------------------------------------------- host prep
def _preprocess(ranks_depth, ranks_feat, ranks_bev):
    ranks_bev = np.asarray(ranks_bev)
    ranks_feat = np.asarray(ranks_feat).astype(np.int64)
    ranks_depth = np.asarray(ranks_depth).astype(np.int64)
    bounds = np.searchsorted(ranks_bev, np.arange(0, NCELLS + 1, CELLS_PER_CORE))
    cores = []
    for k in range(NCORES):
        lo, hi = int(bounds[k]), int(bounds[k + 1])
        rb = ranks_bev[lo:hi].astype(np.int64) - k * CELLS_PER_CORE
        rf = ranks_feat[lo:hi]
        rd = ranks_depth[lo:hi]
        isB = rf >= A_LIM

        # ---------------- stream A ----------------
        rbA, rfA, rdA = rb[~isB], rf[~isB], rd[~isB]
        n = len(rbA)
        assert np.bincount(rbA, minlength=1).max() < 2 * TILE_P
        tiles = []
        s = 0
        while s < n:
            e = min(s + TILE_P, n)
            cut = s + int(np.searchsorted(rbA[s:e], rbA[s] + WIN))
            e = min(e, cut) if cut > s else e
            tiles.append((s, e))
            s = e
        nt = len(tiles)
        assert nt <= T_A, (k, nt)

        rf_i = np.zeros((T_A, TILE_P), np.int64)
        rd_i = np.zeros((T_A, TILE_P), np.int64)
        mask = np.zeros((T_A, TILE_P, WIN), np.float32)
        win0 = np.zeros(T_A, np.int64)
        wid = np.zeros(T_A, np.int64)
        for t, (ts, te) in enumerate(tiles):
            m = te - ts
            rf_i[t, :m] = rfA[ts:te]
            rd_i[t, :m] = rdA[ts:te]
            col = rbA[ts:te] - rbA[ts]
            mask[t, np.arange(m), col] = 1.0
            win0[t] = rbA[ts]
            wid[t] = col[-1] + 1

        rfiA = np.empty((NCHUNK, TILE_P, CHUNK * TILE_P // 16), np.int16)
        rdiA = np.empty_like(rfiA)
        mskA = np.empty((NCHUNK, TILE_P, CHUNK * WIN), np.float32)
        rdmA = np.empty((NCHUNK, TILE_P, CHUNK), np.float32)
        for c in range(NCHUNK):
            t0 = c * CHUNK
            rfiA[c] = _pack16(rf_i[t0:t0 + CHUNK].reshape(-1))
            rdiA[c] = _pack16((rd_i[t0:t0 + CHUNK] // 128).reshape(-1))
            mskA[c] = mask[t0:t0 + CHUNK].transpose(1, 0, 2).reshape(
                TILE_P, CHUNK * WIN)
            rdmA[c] = (rd_i[t0:t0 + CHUNK] % 128).astype(np.float32).T

        # merged scatter: 2 calls per chunk (even/odd tiles), 256 entries
        # each.  Entry i of call `par`: P=i%128, h=i//128,
        # j = 32*h + 2*(P%16) + par, w = P//16.
        sidxA = np.empty((NCHUNK, TILE_P, 2 * 16), np.int16)
        i_arr = np.arange(256)
        P_arr, h_arr = i_arr % 128, i_arr // 128
        w_arr = P_arr // 16
        for c in range(NCHUNK):
            t0 = c * CHUNK
            for par in range(2):
                j = 32 * h_arr + 2 * (P_arr % 16) + par
                t = t0 + j
                ent = np.where(w_arr < wid[t], win0[t] + w_arr,
                               DUMMY + i_arr)
                sidxA[c, :, 16 * par:16 * par + 16] = _pack16(ent)
        # scatter calls are unchained: cells must be globally unique per
        # parity slab (adjacent-tile window overlap always crosses parity)
        for par in range(2):
            j = (32 * h_arr[None, :] + 2 * (P_arr[None, :] % 16) + par
                 + CHUNK * np.arange(NCHUNK)[:, None])
            ent = np.where(w_arr[None, :] < wid[j], win0[j] + w_arr[None, :],
                           DUMMY).ravel()
            live = ent[ent != DUMMY]
            assert len(live) == len(np.unique(live)), (k, par)

        # ---------------- stream B ----------------
        rbB, rfB, rdB = rb[isB], rf[isB], rd[isB]
        nB = len(rbB)
        assert nB <= NB_SLOTS, (k, nB)
        starts = np.concatenate([[0], np.cumsum(B_CAPS)]) * 128  # slot starts
        fill = list(starts[:-1])                  # next free slot per bin
        cell_bins = {}                            # cell -> set of bins used
        slot_of = np.full(NB_SLOTS, -1, np.int64)  # slot -> B-point index
        for i in range(nB):
            cell = int(rbB[i])
            used = cell_bins.setdefault(cell, set())
            placed = False
            for o in range(len(B_CAPS)):
                if o in used or fill[o] >= starts[o + 1]:
                    continue
                slot_of[fill[o]] = i
                fill[o] += 1
                used.add(o)
                placed = True
                break
            assert placed, (k, i, cell)
        rfiB_e = np.zeros(NB_SLOTS, np.int64)
        rdiB_e = np.zeros(NB_SLOTS, np.int64)
        rdmB = np.zeros((TILE_P, NB_BLOCKS), np.float32)
        srowB = DUMMY + (np.arange(NB_SLOTS) % 512)
        occ_s = slot_of >= 0
        pi = slot_of[occ_s]
        rfiB_e[occ_s] = rfB[pi] - A_LIM
        rdiB_e[occ_s] = rdB[pi] // 128
        srowB[occ_s] = rbB[pi]
        g = np.arange(NB_SLOTS)
        rdm_flat = np.zeros(NB_SLOTS, np.float32)
        rdm_flat[occ_s] = (rdB[pi] % 128).astype(np.float32)
        rdmB[g % 128, g // 128] = rdm_flat
        rfiB = _pack16(rfiB_e)
        rdiB = _pack16(rdiB_e)
        sidxB = np.empty((TILE_P, NB_BLOCKS * WIN), np.int16)
        for o in range(len(B_CAPS)):
            s0, e0 = starts[o], starts[o + 1]
            sidxB[:, (s0 // 16):(e0 // 16)] = _pack16(srowB[s0:e0])

        cores.append(dict(rfiA=rfiA, rdiA=rdiA, mskA=_bf16(mskA),
                          rdmA=_bf16(rdmA), sidxA=sidxA, rfiB=rfiB,
                          rdiB=rdiB, rdmB=_bf16(rdmB), sidxB=sidxB))
    return cores


# ---------------------------------------------------------------- program
_CACHED = {}


def _build_program():
    import concourse.bass as bass
    import concourse.bacc as bacc
    import concourse.tile as tile
    from concourse import mybir
    from concourse.tile import add_dep_helper

    nc = bacc.Bacc("TRN2", target_bir_lowering=False, debug=False,
                   num_swdge_queues=4)
    f32, bf16, i16 = mybir.dt.float32, mybir.dt.bfloat16, mybir.dt.int16
    NI = CHUNK * TILE_P                 # 8192 idxs per A chunk
    feat_t = nc.dram_tensor("feat_tbl", [N_FEAT_ROWS, C // 2], f32,
                            kind="ExternalInput").ap()
    dep_t = nc.dram_tensor("dep_tbl", [N_DEP_BLK, 64], f32,
                           kind="ExternalInput").ap()
    iota_t = nc.dram_tensor("iota", [TILE_P, 128], bf16,
                            kind="ExternalInput").ap()
    rfiA_t = nc.dram_tensor("rfiA", [NCHUNK, TILE_P, NI // 16], i16,
                            kind="ExternalInput").ap()
    rdiA_t = nc.dram_tensor("rdiA", [NCHUNK, TILE_P, NI // 16], i16,
                            kind="ExternalInput").ap()
    mskA_t = nc.dram_tensor("mskA", [NCHUNK, TILE_P, CHUNK * WIN], bf16,
                            kind="ExternalInput").ap()
    rdmA_t = nc.dram_tensor("rdmA", [NCHUNK, TILE_P, CHUNK], bf16,
                            kind="ExternalInput").ap()
    sidxA_t = nc.dram_tensor("sidxA", [NCHUNK, TILE_P, 2 * 16], i16,
                             kind="ExternalInput").ap()
    rfiB_t = nc.dram_tensor("rfiB", [TILE_P, NB_SLOTS // 16], i16,
                            kind="ExternalInput").ap()
    rdiB_t = nc.dram_tensor("rdiB", [TILE_P, NB_SLOTS // 16], i16,
                            kind="ExternalInput").ap()
    rdmB_t = nc.dram_tensor("rdmB", [TILE_P, NB_BLOCKS], bf16,
                            kind="ExternalInput").ap()
    sidxB_t = nc.dram_tensor("sidxB", [TILE_P, NB_BLOCKS * WIN], i16,
                             kind="ExternalInput").ap()
    out_t = nc.dram_tensor("out", [CELLS_PER_CORE + 512, C], f32,
                           kind="ExternalOutput").ap()
    out2_t = nc.dram_tensor("out2", [CELLS_PER_CORE + 512, C], f32,
                            kind="ExternalOutput").ap()
    out3_t = nc.dram_tensor("out3", [CELLS_PER_CORE + 512, C], f32,
                            kind="ExternalOutput").ap()

    EQ, MUL, ADD = (mybir.AluOpType.is_equal, mybir.AluOpType.mult,
                    mybir.AluOpType.add)
    AXX = mybir.AxisListType.X

    with tile.TileContext(nc) as tc:
        with (
            tc.tile_pool(name="cst", bufs=1) as cst,
            tc.tile_pool(name="seq", bufs=3) as seq,
            tc.tile_pool(name="gp", bufs=2) as gp,
            tc.tile_pool(name="dp", bufs=3) as dp,
            tc.tile_pool(name="sp", bufs=2) as sp,
            tc.tile_pool(name="xp", bufs=2) as xp,
            tc.tile_pool(name="sip", bufs=NCHUNK) as sip,
            tc.tile_pool(name="sg", bufs=NCHUNK) as sg,
            tc.tile_pool(name="ps", bufs=8, space="PSUM") as ps,
        ):
            iota_sb = cst.tile([TILE_P, 128], bf16)
            nc.sync.dma_start(iota_sb[:], iota_t)

            # ---- stream B gathers first (fills queues 1/2 during warmup)
            rfiB_sb = cst.tile([TILE_P, NB_SLOTS // 16], i16)
            rdiB_sb = cst.tile([TILE_P, NB_SLOTS // 16], i16)
            rdmB_sb = cst.tile([TILE_P, NB_BLOCKS], bf16)
            sidB_sb = cst.tile([TILE_P, NB_BLOCKS * WIN], i16)
            nc.sync.dma_start(rfiB_sb[:], rfiB_t)
            nc.sync.dma_start(rdiB_sb[:], rdiB_t)
            nc.sync.dma_start(rdmB_sb[:], rdmB_t)
            nc.sync.dma_start(sidB_sb[:], sidxB_t)

            gB_sb = cst.tile([TILE_P, NB_BLOCKS * C // 2], f32)
            dbB_sb = cst.tile([TILE_P, NB_BLOCKS * 64], f32)

            # ---- stream A ----
            stg_tiles = []
            prev = None
            for c in range(NCHUNK):
                rfi_sb = seq.tile([TILE_P, NI // 16], i16, tag="rfi")
                rdi_sb = seq.tile([TILE_P, NI // 16], i16, tag="rdi")
                msk_sb = seq.tile([TILE_P, CHUNK * WIN], bf16, tag="msk")
                rdm_sb = seq.tile([TILE_P, CHUNK], bf16, tag="rdm")
                sid_sb = sip.tile([TILE_P, 2 * 16], i16, tag="sid")
                nc.sync.dma_start(rfi_sb[:], rfiA_t[c])
                nc.sync.dma_start(rdi_sb[:], rdiA_t[c])
                nc.sync.dma_start(msk_sb[:], mskA_t[c])
                nc.sync.dma_start(rdm_sb[:], rdmA_t[c])
                nc.sync.dma_start(sid_sb[:], sidxA_t[c])

                g_sb = gp.tile([TILE_P, CHUNK * C // 2], f32, tag="g")
                db_sb = gp.tile([TILE_P, CHUNK * 64], f32, tag="db")
                g3f = g_sb[:].rearrange("p (j e) -> p j e", e=C // 2)
                db3f = db_sb[:].rearrange("p (j e) -> p j e", e=64)
                HN = NI // 2
                nc.gpsimd.dma_gather(g3f[:, :HALF, :], feat_t,
                                     rfi_sb[:, :HN // 16], HN, HN, C // 2,
                                     single_packet=False,
                                     queue_num=c % 4)
                nc.gpsimd.dma_gather(g3f[:, HALF:, :], feat_t,
                                     rfi_sb[:, HN // 16:], HN, HN, C // 2,
                                     single_packet=False,
                                     queue_num=(c + 1) % 4)
                nc.gpsimd.dma_gather(db3f[:, :HALF, :], dep_t,
                                     rdi_sb[:, :HN // 16], HN, HN, 64,
                                     single_packet=False,
                                     queue_num=(c + 2) % 4)
                nc.gpsimd.dma_gather(db3f[:, HALF:, :], dep_t,
                                     rdi_sb[:, HN // 16:], HN, HN, 64,
                                     single_packet=False,
                                     queue_num=(c + 3) % 4)
                if c == 5:
                    gB3f = gB_sb[:].rearrange("p (j e) -> p j e", e=C // 2)
                    dbB3f = dbB_sb[:].rearrange("p (j e) -> p j e", e=64)
                    nc.gpsimd.dma_gather(gB3f, feat_t[A_LIM:, :],
                                         rfiB_sb[:], NB_SLOTS, NB_SLOTS,
                                         C // 2, single_packet=False,
                                         queue_num=(c + 1) % 4)
                if c == 6:
                    nc.gpsimd.dma_gather(dbB3f, dep_t, rdiB_sb[:],
                                         NB_SLOTS, NB_SLOTS, 64,
                                         single_packet=False,
                                         queue_num=(c + 2) % 4)

                g3 = g_sb[:].bitcast(bf16).rearrange("p (j e) -> p j e", e=C)
                db3 = db_sb[:].bitcast(bf16).rearrange(
                    "p (j e) -> p j e", e=128)

                # depth extraction: d[p, j] = db3[p, j, rdm[p, j]]
                d_sb = dp.tile([TILE_P, CHUNK], bf16, tag="d")
                iota3 = iota_sb[:].rearrange("p (j e) -> p j e", j=1) \
                    .to_broadcast([TILE_P, HALF, 128])
                for h in range(2):
                    sel_sb = sp.tile([TILE_P, HALF * 128], bf16, tag="sel")
                    sel3 = sel_sb[:].rearrange("p (j e) -> p j e", e=128)
                    rdm3 = rdm_sb[:, h * HALF:(h + 1) * HALF] \
                        .rearrange("p (j o) -> p j o", o=1) \
                        .to_broadcast([TILE_P, HALF, 128])
                    nc.vector.tensor_tensor(
                        out=sel3, in0=iota3, in1=rdm3, op=EQ)
                    nc.vector.tensor_tensor(
                        out=sel3, in0=sel3,
                        in1=db3[:, h * HALF:(h + 1) * HALF, :], op=MUL)
                    with nc.allow_low_precision(
                            reason="one-hot select, exact"):
                        nc.vector.tensor_reduce(
                            out=d_sb[:, h * HALF:(h + 1) * HALF], in_=sel3,
                            axis=AXX, op=ADD)

                ad_sb = dp.tile([TILE_P, CHUNK * WIN], bf16, tag="ad")
                ad3 = ad_sb[:].rearrange("p (j w) -> p j w", w=WIN)
                nc.vector.tensor_tensor(
                    out=ad3, in0=msk_sb[:].rearrange("p (j w) -> p j w", w=WIN),
                    in1=d_sb[:].to_broadcast([TILE_P, CHUNK, WIN]), op=MUL)

                # staging: element (w, j2, e) of half h lands at stg
                # partition 16w + j2//2, block 2*(j2%2) + h (flat-order DMA
                # pairing); even tiles end up in blocks 0-1, odd in 2-3.
                stg_sb = sg.tile([TILE_P, 4 * C], f32, tag="stg")
                st4 = stg_sb[:].rearrange("P (b e) -> P b e", e=C)
                for h in range(2):
                    tmp_sb = xp.tile([WIN, HALF * C], f32, tag="tmp")
                    for q in range(HALF // 4):
                        pt = ps.tile([WIN, 4 * C], f32, tag="pt", space="PSUM")
                        for m in range(4):
                            j = HALF * h + 4 * q + m
                            nc.tensor.matmul(out=pt[:, C * m:C * (m + 1)],
                                             lhsT=ad3[:, j, :], rhs=g3[:, j, :],
                                             start=True, stop=True)
                        nc.scalar.copy(tmp_sb[:, 4 * C * q:4 * C * (q + 1)],
                                       pt[:])
                    nc.scalar.dma_start(
                        st4[:, h::2, :],
                        tmp_sb[:].rearrange("w (j e) -> w j e", e=C))
                stg_tiles.append((stg_sb, sid_sb))

            # deferred unchained A scatters: even tiles -> out, odd -> out2
            for c in range(NCHUNK):
                stg_sb, sid_sb = stg_tiles[c]
                st4 = stg_sb[:].rearrange("P (b e) -> P b e", e=C)
                for par in range(2):
                    nc.gpsimd.dma_scatter_add(
                        (out_t, out2_t)[par], st4[:, 2 * par:2 * par + 2, :],
                        sid_sb[:, 16 * par:16 * par + 16],
                        256, 256, C, single_packet=False,
                        queue_num=(2 * c + par) % 4)

            # ---- stream B compute + scatter ----
            gB3 = gB_sb[:].bitcast(bf16).rearrange("p (j e) -> p j e", e=C)
            dbB3 = dbB_sb[:].bitcast(bf16).rearrange(
                "p (j e) -> p j e", e=128)
            dB_sb = cst.tile([TILE_P, NB_BLOCKS], bf16)
            iotaB3 = iota_sb[:].rearrange("p (j e) -> p j e", j=1) \
                .to_broadcast([TILE_P, NB_BLOCKS // 2, 128])
            for h in range(2):
                NBH = NB_BLOCKS // 2
                selB_sb = sp.tile([TILE_P, HALF * 128], bf16, tag="sel")
                selB3 = selB_sb[:].rearrange(
                    "p (j e) -> p j e", e=128)[:, :NBH, :]
                rdmB3 = rdmB_sb[:, h * NBH:(h + 1) * NBH] \
                    .rearrange("p (j o) -> p j o", o=1) \
                    .to_broadcast([TILE_P, NBH, 128])
                nc.vector.tensor_tensor(
                    out=selB3, in0=iotaB3, in1=rdmB3, op=EQ)
                nc.vector.tensor_tensor(
                    out=selB3, in0=selB3,
                    in1=dbB3[:, h * NBH:(h + 1) * NBH, :], op=MUL)
                with nc.allow_low_precision(
                        reason="one-hot select, exact"):
                    nc.vector.tensor_reduce(
                        out=dB_sb[:, h * NBH:(h + 1) * NBH], in_=selB3,
                        axis=AXX, op=ADD)
            gsB_sb = cst.tile([TILE_P, NB_BLOCKS * C], f32)
            gsB3 = gsB_sb[:].rearrange("p (j e) -> p j e", e=C)
            nc.vector.tensor_tensor(
                out=gsB3, in0=gB3,
                in1=dB_sb[:].to_broadcast([TILE_P, NB_BLOCKS, C]), op=MUL)

            s0 = 0
            prev = None
            for o, cap in enumerate(B_CAPS):
                e0 = s0 + cap
                sc = nc.gpsimd.dma_scatter_add(
                    out3_t, gsB3[:, s0:e0, :],
                    sidB_sb[:, WIN * s0:WIN * e0],
                    128 * cap, 128 * cap, C, single_packet=False,
                    queue_num=2)
                if prev is not None:
                    add_dep_helper(sc.ins, prev.ins, reason="B scatter chain")
                prev = sc
                s0 = e0
    nc.compile()
    return nc


def _get_program():
    if "nc" not in _CACHED:
        _CACHED["nc"] = _build_program()
    return _CACHED["nc"]


# ---------------------------------------------------------------- entry
def kernel(depth, feat, ranks_depth, ranks_feat, ranks_bev,
           interval_starts=None, interval_lengths=None):
    from concourse import bass_utils

    depth = np.asarray(depth, dtype=np.float32)
    feat = np.asarray(feat, dtype=np.float32)
    feat_flat = _bf16(feat.transpose(0, 1, 3, 4, 2).reshape(-1, C)) \
        .view(np.float32)
    dep_blk = _bf16(depth.reshape(N_DEP_BLK, 128)).view(np.float32)
    iota = _bf16(np.broadcast_to(np.arange(128, dtype=np.float32),
                                 (TILE_P, 128)))

    cores = _preprocess(ranks_depth, ranks_feat, ranks_bev)
    in_maps = []
    for k in range(NCORES):
        cd = cores[k]
        in_maps.append({
            "feat_tbl": feat_flat, "dep_tbl": dep_blk, "iota": iota,
            "rfiA": cd["rfiA"], "rdiA": cd["rdiA"], "mskA": cd["mskA"],
            "rdmA": cd["rdmA"], "sidxA": cd["sidxA"],
            "rfiB": cd["rfiB"], "rdiB": cd["rdiB"], "rdmB": cd["rdmB"],
            "sidxB": cd["sidxB"],
        })

    nc = _get_program()
    res = bass_utils.run_bass_kernel_spmd(nc, in_maps,
                                          core_ids=list(range(NCORES)))
    _CACHED["last_results"] = res

    out_full = np.zeros((B, C, 1, 128, 128), np.float32)
    for k in range(NCORES):
        r = res.results[k]
        oc = (np.asarray(r["out"]) + np.asarray(r["out2"])
              + np.asarray(r["out3"]))[:CELLS_PER_CORE]
        b, blk = k // 4, k % 4
        out_full[b, :, 0, 32 * blk:32 * (blk + 1), :] = \
            oc.T.reshape(C, 32, 128)
    return out_full


# revision 39
# speedup vs baseline: 1.0317x; 1.0130x over previous
"""BevPoolV2 Trainium2 kernel (8 NeuronCores, SPMD, no collectives).

v4: multi-queue SWDGE (gathers spread over Q7 core pairs 0-3), batched
DVE depth extraction (is_equal+mult+reduce over half-chunks instead of
per-tile scalar_tensor_tensor), merged scatters (2 per chunk, even/odd
tiles), deeper tile-pool pipelining, B-stream gathers hoisted ahead of
the A loop.
v3: f32-view gathers (bitcast bf16 for compute).
v2: bf16 gather/matmul path, CHUNK=64, psum groups.
Structure: sorted point stream cut at BEV-cell boundaries; core k owns
cells [4096k, 4096(k+1)); window matmuls accumulate into a [4097, 128]
DRAM slab via chained dma_scatter_add; stream B handles feat rows >=
32768 (int16 gather index limit).
"""
import numpy as np

B, N, D, H, W = 2, 6, 120, 32, 88
C = 128
NCELLS = 32768
NCORES = 8
CELLS_PER_CORE = NCELLS // NCORES   # 4096
TILE_P = 128
WIN = 8
CHUNK = 64                          # tiles per A-chunk
NCHUNK = 15
HALF = 32                           # tiles per psum half-chunk
T_A = NCHUNK * CHUNK                # 960 A-tiles
DUMMY = CELLS_PER_CORE              # trash row 4096
N_FEAT_ROWS = B * N * H * W         # 33792
N_DEPTH = B * N * D * H * W         # 4055040
N_DEP_BLK = N_DEPTH // 128          # 31680
A_LIM = 32768                       # feat rows handled by stream A
NB_ROWS = N_FEAT_ROWS - A_LIM       # 1024 rows in stream-B table view
B_CAPS = (21, 9, 3, 1, 1, 1, 1, 1)  # blocks per B scatter call
NB_BLOCKS = sum(B_CAPS)             # 36
NB_SLOTS = NB_BLOCKS * 128          # 4608


def _pack16(ent):
    """entry i -> int16 storage [i%16, i//16], replicated to 128 partitions."""
    a = np.asarray(ent, np.int16).reshape(-1, 16).T
    return np.ascontiguousarray(np.tile(a, (8, 1)))


def _bf16(x):
    import ml_dtypes
    return np.ascontiguousarray(np.asarray(x).astype(ml_dtypes.bfloat16))


# ---------------------------------------------------------------- host prep
def _preprocess(ranks_depth, ranks_feat, ranks_bev):
    ranks_bev = np.asarray(ranks_bev)
    ranks_feat = np.asarray(ranks_feat).astype(np.int64)
    ranks_depth = np.asarray(ranks_depth).astype(np.int64)
    bounds = np.searchsorted(ranks_bev, np.arange(0, NCELLS + 1, CELLS_PER_CORE))
    cores = []
    for k in range(NCORES):
        lo, hi = int(bounds[k]), int(bounds[k + 1])
        rb = ranks_bev[lo:hi].astype(np.int64) - k * CELLS_PER_CORE
        rf = ranks_feat[lo:hi]
        rd = ranks_depth[lo:hi]
        isB = rf >= A_LIM

        # ---------------- stream A ----------------
        rbA, rfA, rdA = rb[~isB], rf[~isB], rd[~isB]
        n = len(rbA)
        assert np.bincount(rbA, minlength=1).max() < 2 * TILE_P
        tiles = []
        s = 0
        while s < n:
            e = min(s + TILE_P, n)
            cut = s + int(np.searchsorted(rbA[s:e], rbA[s] + WIN))
            e = min(e, cut) if cut > s else e
            tiles.append((s, e))
            s = e
        nt = len(tiles)
        assert nt <= T_A, (k, nt)

        rf_i = np.zeros((T_A, TILE_P), np.int64)
        rd_i = np.zeros((T_A, TILE_P), np.int64)
        mask = np.zeros((T_A, TILE_P, WIN), np.float32)
        win0 = np.zeros(T_A, np.int64)
        wid = np.zeros(T_A, np.int64)
        for t, (ts, te) in enumerate(tiles):
            m = te - ts
            rf_i[t, :m] = rfA[ts:te]
            rd_i[t, :m] = rdA[ts:te]
            col = rbA[ts:te] - rbA[ts]
            mask[t, np.arange(m), col] = 1.0
            win0[t] = rbA[ts]
            wid[t] = col[-1] + 1

        rfiA = np.empty((NCHUNK, TILE_P, CHUNK * TILE_P // 16), np.int16)
        rdiA = np.empty_like(rfiA)
        mskA = np.empty((NCHUNK, TILE_P, CHUNK * WIN), np.float32)
        rdmA = np.empty((NCHUNK, TILE_P, CHUNK), np.float32)
        for c in range(NCHUNK):
            t0 = c * CHUNK
            rfiA[c] = _pack16(rf_i[t0:t0 + CHUNK].reshape(-1))
            rdiA[c] = _pack16((rd_i[t0:t0 + CHUNK] // 128).reshape(-1))
            mskA[c] = mask[t0:t0 + CHUNK].transpose(1, 0, 2).reshape(
                TILE_P, CHUNK * WIN)
            rdmA[c] = (rd_i[t0:t0 + CHUNK] % 128).astype(np.float32).T

        # merged scatter: 2 calls per chunk (even/odd tiles), 256 entries
        # each.  Entry i of call `par`: P=i%128, h=i//128,
        # j = 32*h + 2*(P%16) + par, w = P//16.
        sidxA = np.empty((NCHUNK, TILE_P, 2 * 16), np.int16)
        i_arr = np.arange(256)
        P_arr, h_arr = i_arr % 128, i_arr // 128
        w_arr = P_arr // 16
        for c in range(NCHUNK):
            t0 = c * CHUNK
            for par in range(2):
                j = 32 * h_arr + 2 * (P_arr % 16) + par
                t = t0 + j
                ent = np.where(w_arr < wid[t], win0[t] + w_arr,
                               DUMMY + i_arr)
                sidxA[c, :, 16 * par:16 * par + 16] = _pack16(ent)
        # scatter calls are unchained: cells must be globally unique per
        # parity slab (adjacent-tile window overlap always crosses parity)
        for par in range(2):
            j = (32 * h_arr[None, :] + 2 * (P_arr[None, :] % 16) + par
                 + CHUNK * np.arange(NCHUNK)[:, None])
            ent = np.where(w_arr[None, :] < wid[j], win0[j] + w_arr[None, :],
                           DUMMY).ravel()
            live = ent[ent != DUMMY]
            assert len(live) == len(np.unique(live)), (k, par)

        # ---------------- stream B ----------------
        rbB, rfB, rdB = rb[isB], rf[isB], rd[isB]
        nB = len(rbB)
        assert nB <= NB_SLOTS, (k, nB)
        starts = np.concatenate([[0], np.cumsum(B_CAPS)]) * 128  # slot starts
        fill = list(starts[:-1])                  # next free slot per bin
        cell_bins = {}                            # cell -> set of bins used
        slot_of = np.full(NB_SLOTS, -1, np.int64)  # slot -> B-point index
        for i in range(nB):
            cell = int(rbB[i])
            used = cell_bins.setdefault(cell, set())
            placed = False
            for o in range(len(B_CAPS)):
                if o in used or fill[o] >= starts[o + 1]:
                    continue
                slot_of[fill[o]] = i
                fill[o] += 1
                used.add(o)
                placed = True
                break
            assert placed, (k, i, cell)
        rfiB_e = np.zeros(NB_SLOTS, np.int64)
        rdiB_e = np.zeros(NB_SLOTS, np.int64)
        rdmB = np.zeros((TILE_P, NB_BLOCKS), np.float32)
        srowB = DUMMY + (np.arange(NB_SLOTS) % 512)
        occ_s = slot_of >= 0
        pi = slot_of[occ_s]
        rfiB_e[occ_s] = rfB[pi] - A_LIM
        rdiB_e[occ_s] = rdB[pi] // 128
        srowB[occ_s] = rbB[pi]
        g = np.arange(NB_SLOTS)
        rdm_flat = np.zeros(NB_SLOTS, np.float32)
        rdm_flat[occ_s] = (rdB[pi] % 128).astype(np.float32)
        rdmB[g % 128, g // 128] = rdm_flat
        rfiB = _pack16(rfiB_e)
        rdiB = _pack16(rdiB_e)
        sidxB = np.empty((TILE_P, NB_BLOCKS * WIN), np.int16)
        for o in range(len(B_CAPS)):
            s0, e0 = starts[o], starts[o + 1]
            sidxB[:, (s0 // 16):(e0 // 16)] = _pack16(srowB[s0:e0])

        cores.append(dict(rfiA=rfiA, rdiA=rdiA, mskA=_bf16(mskA),
                          rdmA=_bf16(rdmA), sidxA=sidxA, rfiB=rfiB,
                          rdiB=rdiB, rdmB=_bf16(rdmB), sidxB=sidxB))
    return cores


# ---------------------------------------------------------------- program
_CACHED = {}


def _build_program():
    import concourse.bass as bass
    import concourse.bacc as bacc
    import concourse.tile as tile
    from concourse import mybir
    from concourse.tile import add_dep_helper

    nc = bacc.Bacc("TRN2", target_bir_lowering=False, debug=False,
                   num_swdge_queues=4)
    f32, bf16, i16 = mybir.dt.float32, mybir.dt.bfloat16, mybir.dt.int16
    NI = CHUNK * TILE_P                 # 8192 idxs per A chunk
    feat_t = nc.dram_tensor("feat_tbl", [N_FEAT_ROWS, C // 2], f32,
                            kind="ExternalInput").ap()
    dep_t = nc.dram_tensor("dep_tbl", [N_DEP_BLK, 64], f32,
                           kind="ExternalInput").ap()
    iota_t = nc.dram_tensor("iota", [TILE_P, 128], bf16,
                            kind="ExternalInput").ap()
    rfiA_t = nc.dram_tensor("rfiA", [NCHUNK, TILE_P, NI // 16], i16,
                            kind="ExternalInput").ap()
    rdiA_t = nc.dram_tensor("rdiA", [NCHUNK, TILE_P, NI // 16], i16,
                            kind="ExternalInput").ap()
    mskA_t = nc.dram_tensor("mskA", [NCHUNK, TILE_P, CHUNK * WIN], bf16,
                            kind="ExternalInput").ap()
    rdmA_t = nc.dram_tensor("rdmA", [NCHUNK, TILE_P, CHUNK], bf16,
                            kind="ExternalInput").ap()
    sidxA_t = nc.dram_tensor("sidxA", [NCHUNK, TILE_P, 2 * 16], i16,
                             kind="ExternalInput").ap()
    rfiB_t = nc.dram_tensor("rfiB", [TILE_P, NB_SLOTS // 16], i16,
                            kind="ExternalInput").ap()
    rdiB_t = nc.dram_tensor("rdiB", [TILE_P, NB_SLOTS // 16], i16,
                            kind="ExternalInput").ap()
    rdmB_t = nc.dram_tensor("rdmB", [TILE_P, NB_BLOCKS], bf16,
                            kind="ExternalInput").ap()
    sidxB_t = nc.dram_tensor("sidxB", [TILE_P, NB_BLOCKS * WIN], i16,
                             kind="ExternalInput").ap()
    out_t = nc.dram_tensor("out", [CELLS_PER_CORE + 512, C], f32,
                           kind="ExternalOutput").ap()
    out2_t = nc.dram_tensor("out2", [CELLS_PER_CORE + 512, C], f32,
                            kind="ExternalOutput").ap()
    out3_t = nc.dram_tensor("out3", [CELLS_PER_CORE + 512, C], f32,
                            kind="ExternalOutput").ap()

    EQ, MUL, ADD = (mybir.AluOpType.is_equal, mybir.AluOpType.mult,
                    mybir.AluOpType.add)
    AXX = mybir.AxisListType.X

    with tile.TileContext(nc) as tc:
        with (
            tc.tile_pool(name="cst", bufs=1) as cst,
            tc.tile_pool(name="seq", bufs=3) as seq,
            tc.tile_pool(name="gp", bufs=2) as gp,
            tc.tile_pool(name="dp", bufs=3) as dp,
            tc.tile_pool(name="sp", bufs=2) as sp,
            tc.tile_pool(name="xp", bufs=2) as xp,
            tc.tile_pool(name="sip", bufs=NCHUNK) as sip,
            tc.tile_pool(name="sg", bufs=NCHUNK) as sg,
            tc.tile_pool(name="ps", bufs=8, space="PSUM") as ps,
        ):
            iota_sb = cst.tile([TILE_P, 128], bf16)
            nc.sync.dma_start(iota_sb[:], iota_t)

            # ---- stream B gathers first (fills queues 1/2 during warmup)
            rfiB_sb = cst.tile([TILE_P, NB_SLOTS // 16], i16)
            rdiB_sb = cst.tile([TILE_P, NB_SLOTS // 16], i16)
            rdmB_sb = cst.tile([TILE_P, NB_BLOCKS], bf16)
            sidB_sb = cst.tile([TILE_P, NB_BLOCKS * WIN], i16)
            nc.sync.dma_start(rfiB_sb[:], rfiB_t)
            nc.sync.dma_start(rdiB_sb[:], rdiB_t)
            nc.sync.dma_start(rdmB_sb[:], rdmB_t)
            nc.sync.dma_start(sidB_sb[:], sidxB_t)

            gB_sb = cst.tile([TILE_P, NB_BLOCKS * C // 2], f32)
            dbB_sb = cst.tile([TILE_P, NB_BLOCKS * 64], f32)
            gB3f = gB_sb[:].rearrange("p (j e) -> p j e", e=C // 2)
            dbB3f = dbB_sb[:].rearrange("p (j e) -> p j e", e=64)
            nc.gpsimd.dma_gather(gB3f, feat_t[A_LIM:, :], rfiB_sb[:],
                                 NB_SLOTS, NB_SLOTS, C // 2,
                                 single_packet=False, queue_num=3)
            nc.gpsimd.dma_gather(dbB3f, dep_t, rdiB_sb[:],
                                 NB_SLOTS, NB_SLOTS, 64, single_packet=False,
                                 queue_num=0)

            # ---- stream A ----
            stg_tiles = []
            prev = None
            for c in range(NCHUNK):
                rfi_sb = seq.tile([TILE_P, NI // 16], i16, tag="rfi")
                rdi_sb = seq.tile([TILE_P, NI // 16], i16, tag="rdi")
                msk_sb = seq.tile([TILE_P, CHUNK * WIN], bf16, tag="msk")
                rdm_sb = seq.tile([TILE_P, CHUNK], bf16, tag="rdm")
                sid_sb = sip.tile([TILE_P, 2 * 16], i16, tag="sid")
                nc.sync.dma_start(rfi_sb[:], rfiA_t[c])
                nc.sync.dma_start(rdi_sb[:], rdiA_t[c])
                nc.sync.dma_start(msk_sb[:], mskA_t[c])
                nc.sync.dma_start(rdm_sb[:], rdmA_t[c])
                nc.sync.dma_start(sid_sb[:], sidxA_t[c])

                g_sb = gp.tile([TILE_P, CHUNK * C // 2], f32, tag="g")
                db_sb = gp.tile([TILE_P, CHUNK * 64], f32, tag="db")
                g3f = g_sb[:].rearrange("p (j e) -> p j e", e=C // 2)
                db3f = db_sb[:].rearrange("p (j e) -> p j e", e=64)
                HN = NI // 2
                nc.gpsimd.dma_gather(g3f[:, :HALF, :], feat_t,
                                     rfi_sb[:, :HN // 16], HN, HN, C // 2,
                                     single_packet=False,
                                     queue_num=c % 4)
                nc.gpsimd.dma_gather(g3f[:, HALF:, :], feat_t,
                                     rfi_sb[:, HN // 16:], HN, HN, C // 2,
                                     single_packet=False,
                                     queue_num=(c + 1) % 4)
                nc.gpsimd.dma_gather(db3f[:, :HALF, :], dep_t,
                                     rdi_sb[:, :HN // 16], HN, HN, 64,
                                     single_packet=False,
                                     queue_num=(c + 2) % 4)
                nc.gpsimd.dma_gather(db3f[:, HALF:, :], dep_t,
                                     rdi_sb[:, HN // 16:], HN, HN, 64,
                                     single_packet=False,
                                     queue_num=(c + 3) % 4)
                g3 = g_sb[:].bitcast(bf16).rearrange("p (j e) -> p j e", e=C)
                db3 = db_sb[:].bitcast(bf16).rearrange(
                    "p (j e) -> p j e", e=128)

                # depth extraction: d[p, j] = db3[p, j, rdm[p, j]]
                d_sb = dp.tile([TILE_P, CHUNK], bf16, tag="d")
                iota3 = iota_sb[:].rearrange("p (j e) -> p j e", j=1) \
                    .to_broadcast([TILE_P, HALF, 128])
                for h in range(2):
                    sel_sb = sp.tile([TILE_P, HALF * 128], bf16, tag="sel")
                    sel3 = sel_sb[:].rearrange("p (j e) -> p j e", e=128)
                    rdm3 = rdm_sb[:, h * HALF:(h + 1) * HALF] \
                        .rearrange("p (j o) -> p j o", o=1) \
                        .to_broadcast([TILE_P, HALF, 128])
                    nc.vector.tensor_tensor(
                        out=sel3, in0=iota3, in1=rdm3, op=EQ)
                    nc.vector.tensor_tensor(
                        out=sel3, in0=sel3,
                        in1=db3[:, h * HALF:(h + 1) * HALF, :], op=MUL)
                    with nc.allow_low_precision(
                            reason="one-hot select, exact"):
                        nc.vector.tensor_reduce(
                            out=d_sb[:, h * HALF:(h + 1) * HALF], in_=sel3,
                            axis=AXX, op=ADD)

                ad_sb = dp.tile([TILE_P, CHUNK * WIN], bf16, tag="ad")
                ad3 = ad_sb[:].rearrange("p (j w) -> p j w", w=WIN)
                nc.vector.tensor_tensor(
                    out=ad3, in0=msk_sb[:].rearrange("p (j w) -> p j w", w=WIN),
                    in1=d_sb[:].to_broadcast([TILE_P, CHUNK, WIN]), op=MUL)

                # staging: element (w, j2, e) of half h lands at stg
                # partition 16w + j2//2, block 2*(j2%2) + h (flat-order DMA
                # pairing); even tiles end up in blocks 0-1, odd in 2-3.
                stg_sb = sg.tile([TILE_P, 4 * C], f32, tag="stg")
                st4 = stg_sb[:].rearrange("P (b e) -> P b e", e=C)
                for h in range(2):
                    tmp_sb = xp.tile([WIN, HALF * C], f32, tag="tmp")
                    for q in range(HALF // 4):
                        pt = ps.tile([WIN, 4 * C], f32, tag="pt", space="PSUM")
                        for m in range(4):
                            j = HALF * h + 4 * q + m
                            nc.tensor.matmul(out=pt[:, C * m:C * (m + 1)],
                                             lhsT=ad3[:, j, :], rhs=g3[:, j, :],
                                             start=True, stop=True)
                        nc.scalar.copy(tmp_sb[:, 4 * C * q:4 * C * (q + 1)],
                                       pt[:])
                    nc.scalar.dma_start(
                        st4[:, h::2, :],
                        tmp_sb[:].rearrange("w (j e) -> w j e", e=C))
                stg_tiles.append((stg_sb, sid_sb))

            # deferred unchained A scatters: even tiles -> out, odd -> out2
            for c in range(NCHUNK):
                stg_sb, sid_sb = stg_tiles[c]
                st4 = stg_sb[:].rearrange("P (b e) -> P b e", e=C)
                for par in range(2):
                    nc.gpsimd.dma_scatter_add(
                        (out_t, out2_t)[par], st4[:, 2 * par:2 * par + 2, :],
                        sid_sb[:, 16 * par:16 * par + 16],
                        256, 256, C, single_packet=False,
                        queue_num=(2 * c + par) % 4)

            # ---- stream B compute + scatter ----
            gB3 = gB_sb[:].bitcast(bf16).rearrange("p (j e) -> p j e", e=C)
            dbB3 = dbB_sb[:].bitcast(bf16).rearrange(
                "p (j e) -> p j e", e=128)
            dB_sb = cst.tile([TILE_P, NB_BLOCKS], bf16)
            iotaB3 = iota_sb[:].rearrange("p (j e) -> p j e", j=1) \
                .to_broadcast([TILE_P, NB_BLOCKS // 2, 128])
            for h in range(2):
                NBH = NB_BLOCKS // 2
                selB_sb = sp.tile([TILE_P, HALF * 128], bf16, tag="sel")
                selB3 = selB_sb[:].rearrange(
                    "p (j e) -> p j e", e=128)[:, :NBH, :]
                rdmB3 = rdmB_sb[:, h * NBH:(h + 1) * NBH] \
                    .rearrange("p (j o) -> p j o", o=1) \
                    .to_broadcast([TILE_P, NBH, 128])
                nc.vector.tensor_tensor(
                    out=selB3, in0=iotaB3, in1=rdmB3, op=EQ)
                nc.vector.tensor_tensor(
                    out=selB3, in0=selB3,
                    in1=dbB3[:, h * NBH:(h + 1) * NBH, :], op=MUL)
                with nc.allow_low_precision(
                        reason="one-hot select, exact"):
                    nc.vector.tensor_reduce(
                        out=dB_sb[:, h * NBH:(h + 1) * NBH], in_=selB3,
                        axis=AXX, op=ADD)
            gsB_sb = cst.tile([TILE_P, NB_BLOCKS * C], f32)
            gsB3 = gsB_sb[:].rearrange("p (j e) -> p j e", e=C)
            nc.vector.tensor_tensor(
                out=gsB3, in0=gB3,
                in1=dB_sb[:].to_broadcast([TILE_P, NB_BLOCKS, C]), op=MUL)

            s0 = 0
            prev = None
            for o, cap in enumerate(B_CAPS):
                e0 = s0 + cap
                sc = nc.gpsimd.dma_scatter_add(
                    out3_t, gsB3[:, s0:e0, :],
                    sidB_sb[:, WIN * s0:WIN * e0],
                    128 * cap, 128 * cap, C, single_packet=False,
                    queue_num=2)
                if prev is not None:
                    add_dep_helper(sc.ins, prev.ins, reason="B scatter chain")
                prev = sc
                s0 = e0
    nc.compile()
    return nc


def _get_program():
    if "nc" not in _CACHED:
        _CACHED["nc"] = _build_program()
    return _CACHED["nc"]


# ---------------------------------------------------------------- entry
def kernel(depth, feat, ranks_depth, ranks_feat, ranks_bev,
           interval_starts=None, interval_lengths=None):
    from concourse import bass_utils

    depth = np.asarray(depth, dtype=np.float32)
    feat = np.asarray(feat, dtype=np.float32)
    feat_flat = _bf16(feat.transpose(0, 1, 3, 4, 2).reshape(-1, C)) \
        .view(np.float32)
    dep_blk = _bf16(depth.reshape(N_DEP_BLK, 128)).view(np.float32)
    iota = _bf16(np.broadcast_to(np.arange(128, dtype=np.float32),
                                 (TILE_P, 128)))

    cores = _preprocess(ranks_depth, ranks_feat, ranks_bev)
    in_maps = []
    for k in range(NCORES):
        cd = cores[k]
        in_maps.append({
            "feat_tbl": feat_flat, "dep_tbl": dep_blk, "iota": iota,
            "rfiA": cd["rfiA"], "rdiA": cd["rdiA"], "mskA": cd["mskA"],
            "rdmA": cd["rdmA"], "sidxA": cd["sidxA"],
            "rfiB": cd["rfiB"], "rdiB": cd["rdiB"], "rdmB": cd["rdmB"],
            "sidxB": cd["sidxB"],
        })

    nc = _get_program()
    res = bass_utils.run_bass_kernel_spmd(nc, in_maps,
                                          core_ids=list(range(NCORES)))
    _CACHED["last_results"] = res

    out_full = np.zeros((B, C, 1, 128, 128), np.float32)
    for k in range(NCORES):
        r = res.results[k]
        oc = (np.asarray(r["out"]) + np.asarray(r["out2"])
              + np.asarray(r["out3"]))[:CELLS_PER_CORE]
        b, blk = k // 4, k % 4
        out_full[b, :, 0, 32 * blk:32 * (blk + 1), :] = \
            oc.T.reshape(C, 32, 128)
    return out_full
